# revision 31
# baseline (speedup 1.0000x reference)
"""AttentionPooling (segment softmax pooling) on 8 Trainium2 NeuronCores.

z[b] = sum_i softmax_within_segment(alpha)_i * x_i  for segment b, where
alpha = tanh(x @ W1.T) @ W2.T.

Strategy (data parallel over segments):
- batch is sorted, B = 1024 = 8 * 128, so core c owns segments
  [128c, 128(c+1)) — a contiguous row range of x. No cross-core segments,
  so the host just concatenates per-core results.
- alpha for this distribution lies in ~[-3, 3], so exp() without the
  per-segment max subtraction is numerically safe; softmax = e / seg_sum(e).
- HBM traffic is the bottleneck, so x ships once in fp16 (row-major, the
  value path) and once in fp8e4 (transposed, feeds only the attention
  logits — fp8 noise there only perturbs softmax weights slightly;
  measured end-to-end rel err 1.7e-2 < 2e-2).
- mm1 runs as TWO DoubleRow fp8 matmuls accumulating into one PSUM:
  W1 ships as A = fp8(16*W1) plus the residual B = fp8(16*W1 - A), so W1
  itself contributes only ~bf16-level error; tanh applies scale=1/16.
  DoubleRow contracts K=256 in one pass.
- Per 128-row tile on device:
    yT   = (A + B) @ x_tile.T     (PE, 2 fp8 DoubleRow matmuls per quad)
    th   = tanh(yT / 16)          (ACT, PSUM->SBUF fp16)
    a    = th.T @ W2              (PE -> (128 rows x 1) PSUM per chunk)
    e    = exp(a)                 (ACT, batched per half-group)
    E    = (iota == colidx%16)*e  (DVE, built for a whole half-group per
                                   op via stride-0 broadcast APs)
    gpool += E.T @ x_tile         (PE, (16 x 256) per-group PSUM; a
                                   31-tile group spans <= 10 segments so
                                   mod-16 is collision-free within a group)
  ...and once per 31-tile group:
    pool += scatter_g.T @ gpool   (PE, fp16 0/1 scatter matmul - exact -
                                   into the persistent (128 segs x 256) pool)
- The PE executes in order, so consumers are software-pipelined behind
  their producers: the a = th.T @ W2 matmuls of a half-group are emitted
  one half-group late (their tanh has retired by then), the pool matmuls
  of a group run as one 31-matmul batch two half-groups after its last E
  was built (covering the ACT exp -> DVE E chain), and each scatter
  matmul is owed until the next group's pool batch (covering the DVE
  gp drain). Inputs prefetch 5 groups deep; within a group the fp8 xT
  DMA precedes the fp16 x DMA because mm1 consumes first and the input
  queue drains strictly in emission order.
- Host: denominator d_s = segment_sum(fp16(e)) from the e dump (exactly
  the same fp16 values the E matrix used), z = pool / d.
"""

import numpy as np
import ml_dtypes

import concourse.bacc as bacc
import concourse.mybir as mybir
import concourse.tile as tile
from concourse.bass_utils import run_bass_kernel_spmd

f16 = np.float16
f8 = ml_dtypes.float8_e4m3
F32 = mybir.dt.float32
F16 = mybir.dt.float16
F8E4 = mybir.dt.float8e4
AF = mybir.ActivationFunctionType
ALU = mybir.AluOpType
SWI = True  # DoubleRowSwInterleave (pre-interleaved weights) vs DoubleRow
DR = (mybir.MatmulPerfMode.DoubleRowSwInterleave if SWI
      else mybir.MatmulPerfMode.DoubleRow)

NCORES = 8
D = 256
H = 128
SEGS_PER_CORE = 128
GT = 31          # tiles per DMA group; a 31-tile group spans <= ~10 segments
EW = 16          # one-hot width: local segment index mod EW within a group
W1_SCALE = 16.0  # W1 pre-scale so fp8(16*W1) stays in normal range

# (start, len) of the half-group batches inside a group
HALves = [(0, 16), (16, 15)]

_kernel_cache = {}


def _build_kernel(nt):
    """Build + compile the per-core SPMD kernel for nt 128-row tiles."""
    assert nt % GT == 0
    ngroups = nt // GT
    nc = bacc.Bacc("TRN2", target_bir_lowering=False, debug=False)

    x_nat_d = nc.dram_tensor("x_nat", [128, nt, D], F16, kind="ExternalInput").ap()
    # per-group-contiguous: one 2*GT*128-byte line per partition per group
    xt_d = nc.dram_tensor("xT", [128, ngroups, 2, GT * 128], F8E4,
                          kind="ExternalInput").ap()
    ci_d = nc.dram_tensor("colidx", [128, nt], F16, kind="ExternalInput").ap()
    w1_shape = [128, 2 * H] if SWI else [128, 2, H]
    w1a_d = nc.dram_tensor("W1A", w1_shape, F8E4, kind="ExternalInput").ap()
    w1b_d = nc.dram_tensor("W1B", w1_shape, F8E4, kind="ExternalInput").ap()
    w2_d = nc.dram_tensor("W2c", [H, 1], F16, kind="ExternalInput").ap()
    iota_d = nc.dram_tensor("iota", [128, EW], F16, kind="ExternalInput").ap()
    scat_d = nc.dram_tensor("scat", [EW, ngroups, SEGS_PER_CORE], F16,
                            kind="ExternalInput").ap()
    out_d = nc.dram_tensor("out", [SEGS_PER_CORE, D], F32, kind="ExternalOutput").ap()
    e_out_d = nc.dram_tensor("e_out", [128, nt], F16, kind="ExternalOutput").ap()

    with tile.TileContext(nc) as tc:
        with (
            tc.tile_pool(name="const", bufs=1) as constp,
            tc.tile_pool(name="xn", bufs=5) as xnp,
            tc.tile_pool(name="xt", bufs=5) as xtp,
            tc.tile_pool(name="th", bufs=8) as thp,
            tc.tile_pool(name="ee", bufs=8) as eep,
            tc.tile_pool(name="gps", bufs=3) as gpsp,
            tc.tile_pool(name="out", bufs=1) as outp,
            tc.tile_pool(name="psum_y", bufs=4, space="PSUM") as psumy,
            tc.tile_pool(name="psum_al", bufs=2, space="PSUM") as psumal,
            tc.tile_pool(name="psum_gp", bufs=1, space="PSUM") as psumgp,
            tc.tile_pool(name="psum_acc", bufs=1, space="PSUM") as psumacc,
        ):
            w1a_sb = constp.tile(w1_shape, F8E4)
            nc.default_dma_engine.dma_start(w1a_sb[:], w1a_d[:])
            w1b_sb = constp.tile(w1_shape, F8E4)
            nc.default_dma_engine.dma_start(w1b_sb[:], w1b_d[:])
            w2_sb = constp.tile([H, 1], F16)
            nc.default_dma_engine.dma_start(w2_sb[:], w2_d[:])
            iota_sb = constp.tile([128, EW], F16)
            ci_sb = constp.tile([128, nt], F16)
            scat_sb = constp.tile([EW, ngroups, SEGS_PER_CORE], F16)
            e_buf = constp.tile([128, nt], F16)

            pool_ps = psumacc.tile([SEGS_PER_CORE, D], F32)

            state = {}     # g -> (xn, xt, gp_ps, al_ps)
            E8s = {}       # g -> [(s, l, E8), ...]
            ready_pool = []  # groups whose E matrices are all built
            scat_q = []    # scatter matmuls owed, emitted one group late
            pend_mm2 = None  # (g, s, l, ths) whose mm2/exp/E are owed

            def enter_group(g):
                gstart = g * GT
                # xt first: mm1 consumes it first, and the input queue
                # drains strictly in emission order
                xt = xtp.tile([128, 2, GT * 128], F8E4, tag="xt")
                nc.default_dma_engine.dma_start(xt[:], xt_d[:, g, :, :])
                xn = xnp.tile([128, GT, D], F16, tag="xn")
                nc.default_dma_engine.dma_start(
                    xn[:], x_nat_d[:, gstart:gstart + GT, :])
                al_ps = psumal.tile([128, GT], F32, tag="al")
                state[g] = (xn, xt, al_ps)

            def emit_scat(flush=False):
                while scat_q and (flush or len(scat_q) > 1):
                    sg, gp_sb = scat_q.pop(0)
                    nc.tensor.matmul(pool_ps[:], scat_sb[:, sg, :], gp_sb[:],
                                     start=(sg == 0), stop=(sg == ngroups - 1))

            def emit_mm2_and_E(p):
                """mm2 + exp + E-build for a half-group, one half late: by
                now its tanh outputs have long retired, so the PE never
                stalls on th loads."""
                g, s, l, ths = p
                _, _, al_ps = state[g]
                for q0, ql, th in ths:
                    for j in range(ql):
                        c = q0 + j
                        nc.tensor.matmul(al_ps[:, c:c + 1],
                                         th[:, j * 128:(j + 1) * 128],
                                         w2_sb[:], start=True, stop=True)
                t0 = g * GT + s
                nc.scalar.activation(e_buf[:, t0:t0 + l],
                                     al_ps[:, s:s + l], AF.Exp)
                S8 = eep.tile([128, l, EW], F16, tag="S8",
                              padded_shape=[128, 16, EW])
                nc.vector.tensor_tensor(
                    S8[:],
                    ci_sb[:, t0:t0 + l].broadcast_to([128, l, EW]),
                    iota_sb[:, None, :].broadcast_to([128, l, EW]),
                    ALU.is_equal)
                E8 = eep.tile([128, l, EW], F16, tag="E8",
                              padded_shape=[128, 16, EW])
                nc.vector.tensor_mul(
                    E8[:], S8[:],
                    e_buf[:, t0:t0 + l].broadcast_to([128, l, EW]))
                E8s.setdefault(g, []).append((s, l, E8))
                if s + l == GT:
                    ready_pool.append(g)

            def emit_group_pool(g):
                """All 31 pool matmuls of a group back-to-back (one batch
                overhead), paced two halves after its E matrices closed."""
                xn, _, _ = state[g]
                gp_ps = psumgp.tile([EW, D], F32, tag="gp")
                for s, l, E8 in E8s.pop(g):
                    for j in range(l):
                        tg = s + j
                        nc.tensor.matmul(gp_ps[:], E8[:, j, :], xn[:, tg, :],
                                         start=(tg == 0), stop=(tg == GT - 1))
                emit_scat()
                gp_sb = gpsp.tile([EW, D], F16, tag="gp_sb")
                nc.vector.tensor_copy(gp_sb[:], gp_ps[:])
                scat_q.append((g, gp_sb))
                del state[g]

            for g in range(ngroups):
                enter_group(g)
                _, xt, _ = state[g]
                if g == 0:
                    # small consts ride the queue behind group 0's data;
                    # none are needed before the first E-build / scatter
                    nc.default_dma_engine.dma_start(iota_sb[:], iota_d[:])
                    nc.default_dma_engine.dma_start(ci_sb[:], ci_d[:])
                    nc.default_dma_engine.dma_start(scat_sb[:], scat_d[:])
                for s, l in HALves:
                    # mm1 + tanh for the half-group, in quads
                    ths = []
                    for q0, ql in ((s, 4), (s + 4, 4), (s + 8, 4),
                                   (s + 12, l - 12)):
                        y_ps = psumy.tile([128, ql * 128], F32, tag="y",
                                          padded_shape=[128, 512])
                        xt_q = xt[:, :, q0 * 128:(q0 + ql) * 128]
                        nc.tensor.matmul(y_ps[:], w1a_sb[:], xt_q,
                                         start=True, stop=False, perf_mode=DR)
                        nc.tensor.matmul(y_ps[:], w1b_sb[:], xt_q,
                                         start=False, stop=True, perf_mode=DR)
                        th = thp.tile([128, ql * 128], F16, tag="th",
                                      padded_shape=[128, 512])
                        nc.scalar.activation(th[:], y_ps[:], AF.Tanh,
                                             scale=1.0 / W1_SCALE)
                        ths.append((q0, ql, th))

                    if pend_mm2 is not None:
                        emit_mm2_and_E(pend_mm2)
                    pend_mm2 = (g, s, l, ths)

                    if g == ngroups // 2 and s == 0:
                        # first half of the e dump (tiles of groups
                        # 0..mid-1 are final), overlapped with compute.
                        # Issued from the ACT sequencer: on the Sync/DGE
                        # queue its wait-for-exp would head-of-line block
                        # the input DMA issues behind it.
                        t = (ngroups // 2) * GT
                        nc.scalar.dma_start(e_out_d[:, :t], e_buf[:, :t])
                    if s == 16 and ready_pool:
                        emit_group_pool(ready_pool.pop(0))

            emit_mm2_and_E(pend_mm2)
            while ready_pool:
                emit_group_pool(ready_pool.pop(0))
            emit_scat(flush=True)

            pool_sb = outp.tile([SEGS_PER_CORE, D], F32)
            nc.scalar.activation(pool_sb[:], pool_ps[:], AF.Copy)
            nc.default_dma_engine.dma_start(out_d[:], pool_sb[:])
            t = (ngroups // 2) * GT
            nc.default_dma_engine.dma_start(e_out_d[:, t:], e_buf[:, t:])

    nc.compile()
    return nc


def _prep_core(x, batch, r0, r1, seg0, nt):
    """Host-side shard prep for one core: rows [r0, r1) own segments
    [seg0, seg0+128). Returns the per-core input map."""
    rows = r1 - r0
    pad_rows = nt * 128

    xb = np.zeros((pad_rows, D), dtype=f16)
    xb[:rows] = x[r0:r1].astype(f16)
    # (128, nt, D): partition p holds row t*128 + p
    x_nat = np.ascontiguousarray(xb.reshape(nt, 128, D).transpose(1, 0, 2))

    x8 = np.zeros((pad_rows, D), dtype=f8)
    x8[:rows] = x[r0:r1].astype(f8)
    # (128, ngroups, 2, GT*128): partition d' holds feature c*128 + d',
    # packed so each group's slice is one contiguous line per partition
    xT = np.ascontiguousarray(
        x8.T.reshape(2, 128, nt // GT, GT * 128).transpose(1, 2, 0, 3))

    seg_local = np.full(pad_rows, -1, dtype=np.int64)
    seg_local[:rows] = batch[r0:r1] - seg0
    ci = np.where(seg_local < 0, -1.0, seg_local % EW).astype(np.float32)
    colidx = np.ascontiguousarray(ci.reshape(nt, 128).T).astype(f16)  # (128, nt)

    # scatter matrices: scat[k, g, s] = 1 iff group g's pool row k holds
    # local segment s (k = s mod EW). A 31-tile group spans <= ~10
    # consecutive segments, so within a group mod-EW is collision free.
    ngroups = nt // GT
    scat = np.zeros((EW, ngroups, SEGS_PER_CORE), dtype=f16)
    for g in range(ngroups):
        segs = np.unique(seg_local[g * GT * 128:(g + 1) * GT * 128])
        segs = segs[segs >= 0]
        assert segs.size <= EW, f"group {g} spans {segs.size} segments > EW"
        scat[segs % EW, g, segs] = 1.0

    return {"x_nat": x_nat, "xT": xT, "colidx": colidx, "scat": scat}


def _shared_inputs(W1, W2):
    W1s = (W1_SCALE * W1).astype(np.float32)
    A = W1s.astype(f8)
    Bm = (W1s - A.astype(np.float32)).astype(f8)

    if SWI:
        def pack(w):
            # (H, D) -> (128, 2H) flat: flat[p, 2*(127-h)+c] = w[h, c*128+p]
            Wc = np.asarray(w).T.reshape(2, 128, H)  # [c, p, h]
            flat = np.zeros((128, 2 * H), dtype=w.dtype)
            h = np.arange(H)
            for c in range(2):
                flat[:, 2 * (H - 1 - h) + c] = Wc[c]
            return np.ascontiguousarray(flat)
    else:
        def pack(w):  # (H, D) -> (128, 2, H) with [d', c, h] = w[h, c*128+d']
            return np.ascontiguousarray(
                np.asarray(w).T.reshape(2, 128, H).transpose(1, 0, 2))

    w2c = np.ascontiguousarray(W2.reshape(H, 1).astype(f16))
    iota = np.broadcast_to(
        np.arange(EW, dtype=np.float32), (128, EW)).astype(f16)
    return {"W1A": pack(A), "W1B": pack(Bm), "W2c": w2c, "iota": iota}


def _seg_starts(x, batch):
    s = np.searchsorted(batch, np.arange(0, NCORES * SEGS_PER_CORE + 1, SEGS_PER_CORE))
    s[0], s[-1] = 0, x.shape[0]
    return s


def build_in_maps(x, batch, nt):
    s = _seg_starts(x, batch)
    return [_prep_core(x, batch, int(s[c]), int(s[c + 1]), c * SEGS_PER_CORE, nt)
            for c in range(NCORES)]


def pick_nt(x, batch):
    s = _seg_starts(x, batch)
    nt = int(max(-(-(int(s[c + 1] - s[c])) // 128) for c in range(NCORES)))
    return -(-nt // GT) * GT


def kernel(x, batch, W1, W2, B):
    x = np.asarray(x)
    batch = np.asarray(batch)
    W1 = np.asarray(W1)
    W2 = np.asarray(W2)
    B = int(B)
    assert B == NCORES * SEGS_PER_CORE

    nt = pick_nt(x, batch)
    if nt not in _kernel_cache:
        _kernel_cache[nt] = _build_kernel(nt)
    nc = _kernel_cache[nt]

    shared = _shared_inputs(W1, W2)
    in_maps = build_in_maps(x, batch, nt)
    for m in in_maps:
        m.update(shared)

    res = run_bass_kernel_spmd(nc, in_maps, core_ids=list(range(NCORES)))

    seg_starts = _seg_starts(x, batch)
    z = np.empty((B, D), dtype=np.float32)
    for c in range(NCORES):
        num = res.results[c]["out"]  # (128, D)
        # denominator from the e dump, rounded exactly like the E matrix
        e = res.results[c]["e_out"].T.reshape(-1)  # row t*128+p -> e
        r0, r1 = int(seg_starts[c]), int(seg_starts[c + 1])
        seg_local = (batch[r0:r1] - c * SEGS_PER_CORE).astype(np.int64)
        e_rows = e[:r1 - r0].astype(np.float64)
        den = np.bincount(seg_local, weights=e_rows, minlength=SEGS_PER_CORE)
        den = np.where(den == 0.0, 1.0, den).astype(np.float32)
        z[c * SEGS_PER_CORE:(c + 1) * SEGS_PER_CORE] = num / den[:, None]
    return z


# revision 32
# speedup vs baseline: 1.1957x; 1.1957x over previous
"""AttentionPooling (segment softmax pooling) on 8 Trainium2 NeuronCores.

z[b] = sum_i softmax_within_segment(alpha)_i * x_i  for segment b, where
alpha = tanh(x @ W1.T) @ W2.T.

Strategy (data parallel over segments):
- batch is sorted, B = 1024 = 8 * 128, so core c owns segments
  [128c, 128(c+1)) — a contiguous row range of x. No cross-core segments,
  so the host just concatenates per-core results.
- alpha for this distribution lies in ~[-3, 3], so exp() without the
  per-segment max subtraction is numerically safe; softmax = e / seg_sum(e).
- HBM traffic is the bottleneck, so x ships once in fp16 (row-major, the
  value path) and once in fp8e4 (transposed, feeds only the attention
  logits — fp8 noise there only perturbs softmax weights slightly;
  measured end-to-end rel err 1.7e-2 < 2e-2).
- mm1 runs as TWO DoubleRow fp8 matmuls accumulating into one PSUM:
  W1 ships as A = fp8(16*W1) plus the residual B = fp8(16*W1 - A), so W1
  itself contributes only ~bf16-level error; tanh applies scale=1/16.
  DoubleRow contracts K=256 in one pass.
- Per 128-row tile on device:
    yT   = (A + B) @ x_tile.T     (PE, 2 fp8 DoubleRow matmuls per quad)
    th   = tanh(yT / 16)          (ACT, PSUM->SBUF fp16)
    a    = th.T @ W2              (PE -> (128 rows x 1) PSUM per chunk)
    e    = exp(a)                 (ACT, batched per half-group)
    E    = (iota == colidx%16)*e  (DVE, built for a whole half-group per
                                   op via stride-0 broadcast APs)
    gpool += E.T @ x_tile         (PE, (16 x 256) per-group PSUM; a
                                   31-tile group spans <= 10 segments so
                                   mod-16 is collision-free within a group)
  ...and once per 31-tile group:
    pool += scatter_g.T @ gpool   (PE, fp16 0/1 scatter matmul - exact -
                                   into the persistent (128 segs x 256) pool)
- The PE executes in order, so consumers are software-pipelined behind
  their producers: the a = th.T @ W2 matmuls of a half-group are emitted
  one half-group late (their tanh has retired by then), the pool matmuls
  of a group run as one 31-matmul batch two half-groups after its last E
  was built (covering the ACT exp -> DVE E chain), and each scatter
  matmul is owed until the next group's pool batch (covering the DVE
  gp drain). Inputs prefetch 5 groups deep; within a group the fp8 xT
  DMA precedes the fp16 x DMA because mm1 consumes first and the input
  queue drains strictly in emission order.
- Host: denominator d_s = segment_sum(fp16(e)) from the e dump (exactly
  the same fp16 values the E matrix used), z = pool / d.
"""

import numpy as np
import ml_dtypes

import concourse.bacc as bacc
import concourse.mybir as mybir
import concourse.tile as tile
from concourse.bass_utils import run_bass_kernel_spmd

f16 = np.float16
f8 = ml_dtypes.float8_e4m3
F32 = mybir.dt.float32
F16 = mybir.dt.float16
F8E4 = mybir.dt.float8e4
AF = mybir.ActivationFunctionType
ALU = mybir.AluOpType
SWI = True  # DoubleRowSwInterleave (pre-interleaved weights) vs DoubleRow
DR = (mybir.MatmulPerfMode.DoubleRowSwInterleave if SWI
      else mybir.MatmulPerfMode.DoubleRow)

NCORES = 8
D = 256
H = 128
SEGS_PER_CORE = 128
GT = 31          # tiles per DMA group; a 31-tile group spans <= ~10 segments
EW = 16          # one-hot width: local segment index mod EW within a group
W1_SCALE = 16.0  # W1 pre-scale so fp8(16*W1) stays in normal range

# (start, len) of the half-group batches inside a group
HALves = [(0, 8), (8, 8), (16, 8), (24, 7)]

_kernel_cache = {}


def _build_kernel(nt):
    """Build + compile the per-core SPMD kernel for nt 128-row tiles."""
    assert nt % GT == 0
    ngroups = nt // GT
    nc = bacc.Bacc("TRN2", target_bir_lowering=False, debug=False)

    x_nat_d = nc.dram_tensor("x_nat", [128, nt, D], F16, kind="ExternalInput").ap()
    # per-group-contiguous: one 2*GT*128-byte line per partition per group
    xt_d = nc.dram_tensor("xT", [128, ngroups, 2, GT * 128], F8E4,
                          kind="ExternalInput").ap()
    ci_d = nc.dram_tensor("colidx", [128, nt], F16, kind="ExternalInput").ap()
    w1_shape = [128, 2 * H] if SWI else [128, 2, H]
    w1a_d = nc.dram_tensor("W1A", w1_shape, F8E4, kind="ExternalInput").ap()
    w1b_d = nc.dram_tensor("W1B", w1_shape, F8E4, kind="ExternalInput").ap()
    w2_d = nc.dram_tensor("W2c", [H, 1], F16, kind="ExternalInput").ap()
    iota_d = nc.dram_tensor("iota", [128, EW], F16, kind="ExternalInput").ap()
    scat_d = nc.dram_tensor("scat", [EW, ngroups, SEGS_PER_CORE], F16,
                            kind="ExternalInput").ap()
    out_d = nc.dram_tensor("out", [SEGS_PER_CORE, D], F32, kind="ExternalOutput").ap()
    e_out_d = nc.dram_tensor("e_out", [128, nt], F16, kind="ExternalOutput").ap()

    with tile.TileContext(nc) as tc:
        with (
            tc.tile_pool(name="const", bufs=1) as constp,
            tc.tile_pool(name="xn", bufs=5) as xnp,
            tc.tile_pool(name="xt", bufs=5) as xtp,
            tc.tile_pool(name="th", bufs=6) as thp,
            tc.tile_pool(name="ee", bufs=8) as eep,
            tc.tile_pool(name="gps", bufs=3) as gpsp,
            tc.tile_pool(name="out", bufs=1) as outp,
            tc.tile_pool(name="psum_y", bufs=3, space="PSUM") as psumy,
            tc.tile_pool(name="psum_al", bufs=2, space="PSUM") as psumal,
            tc.tile_pool(name="psum_gp", bufs=2, space="PSUM") as psumgp,
            tc.tile_pool(name="psum_acc", bufs=1, space="PSUM") as psumacc,
        ):
            w1a_sb = constp.tile(w1_shape, F8E4)
            nc.default_dma_engine.dma_start(w1a_sb[:], w1a_d[:])
            w1b_sb = constp.tile(w1_shape, F8E4)
            nc.default_dma_engine.dma_start(w1b_sb[:], w1b_d[:])
            w2_sb = constp.tile([H, 1], F16)
            nc.default_dma_engine.dma_start(w2_sb[:], w2_d[:])
            iota_sb = constp.tile([128, EW], F16)
            ci_sb = constp.tile([128, nt], F16)
            scat_sb = constp.tile([EW, ngroups, SEGS_PER_CORE], F16)
            e_buf = constp.tile([128, nt], F16)

            pool_ps = psumacc.tile([SEGS_PER_CORE, D], F32)

            state = {}     # g -> (xn, xt, gp_ps, al_ps)
            E8s = {}       # g -> [(s, l, E8), ...]
            ready_pool = []  # groups whose E matrices are all built
            scat_q = []    # scatter matmuls owed, emitted one group late
            pend_mm2 = None  # (g, s, l, ths) whose mm2/exp/E are owed

            def enter_group(g):
                gstart = g * GT
                # xt first: mm1 consumes it first, and the input queue
                # drains strictly in emission order
                xt = xtp.tile([128, 2, GT * 128], F8E4, tag="xt")
                nc.default_dma_engine.dma_start(xt[:], xt_d[:, g, :, :])
                xn = xnp.tile([128, GT, D], F16, tag="xn")
                nc.default_dma_engine.dma_start(
                    xn[:], x_nat_d[:, gstart:gstart + GT, :])
                al_ps = psumal.tile([128, GT], F32, tag="al")
                state[g] = (xn, xt, al_ps)

            def emit_scat(flush=False):
                while scat_q and (flush or len(scat_q) > 1):
                    sg, gp_sb = scat_q.pop(0)
                    nc.tensor.matmul(pool_ps[:], scat_sb[:, sg, :], gp_sb[:],
                                     start=(sg == 0), stop=(sg == ngroups - 1))

            def emit_mm2_and_E(p):
                """mm2 + exp + E-build for a half-group, one half late: by
                now its tanh outputs have long retired, so the PE never
                stalls on th loads."""
                g, s, l, ths = p
                _, _, al_ps = state[g]
                for q0, ql, th in ths:
                    for j in range(ql):
                        c = q0 + j
                        nc.tensor.matmul(al_ps[:, c:c + 1],
                                         th[:, j * 128:(j + 1) * 128],
                                         w2_sb[:], start=True, stop=True)
                t0 = g * GT + s
                nc.scalar.activation(e_buf[:, t0:t0 + l],
                                     al_ps[:, s:s + l], AF.Exp)
                S8 = eep.tile([128, l, EW], F16, tag="S8",
                              padded_shape=[128, 8, EW])
                nc.vector.tensor_tensor(
                    S8[:],
                    ci_sb[:, t0:t0 + l].broadcast_to([128, l, EW]),
                    iota_sb[:, None, :].broadcast_to([128, l, EW]),
                    ALU.is_equal)
                E8 = eep.tile([128, l, EW], F16, tag="E8",
                              padded_shape=[128, 8, EW])
                nc.vector.tensor_mul(
                    E8[:], S8[:],
                    e_buf[:, t0:t0 + l].broadcast_to([128, l, EW]))
                E8s.setdefault(g, []).append((s, l, E8))
                if s + l == GT:
                    ready_pool.append(g)

            def emit_group_pool(g):
                """All 31 pool matmuls of a group back-to-back (one batch
                overhead), paced two halves after its E matrices closed."""
                xn, _, _ = state[g]
                gp_ps = psumgp.tile([EW, D], F32, tag="gp")
                for s, l, E8 in E8s.pop(g):
                    for j in range(l):
                        tg = s + j
                        nc.tensor.matmul(gp_ps[:], E8[:, j, :], xn[:, tg, :],
                                         start=(tg == 0), stop=(tg == GT - 1))
                emit_scat()
                gp_sb = gpsp.tile([EW, D], F16, tag="gp_sb")
                nc.vector.tensor_copy(gp_sb[:], gp_ps[:])
                scat_q.append((g, gp_sb))
                del state[g]

            for g in range(ngroups):
                enter_group(g)
                _, xt, _ = state[g]
                if g == 0:
                    # small consts ride the queue behind group 0's data;
                    # none are needed before the first E-build / scatter
                    nc.default_dma_engine.dma_start(iota_sb[:], iota_d[:])
                    nc.default_dma_engine.dma_start(ci_sb[:], ci_d[:])
                    nc.default_dma_engine.dma_start(scat_sb[:], scat_d[:])
                for s, l in HALves:
                    # mm1 + tanh for the half-group, in quads
                    ths = []
                    for q0, ql in ((s, 4), (s + 4, l - 4)):
                        y_ps = psumy.tile([128, ql * 128], F32, tag="y",
                                          padded_shape=[128, 512])
                        xt_q = xt[:, :, q0 * 128:(q0 + ql) * 128]
                        nc.tensor.matmul(y_ps[:], w1a_sb[:], xt_q,
                                         start=True, stop=False, perf_mode=DR)
                        nc.tensor.matmul(y_ps[:], w1b_sb[:], xt_q,
                                         start=False, stop=True, perf_mode=DR)
                        th = thp.tile([128, ql * 128], F16, tag="th",
                                      padded_shape=[128, 512])
                        nc.scalar.activation(th[:], y_ps[:], AF.Tanh,
                                             scale=1.0 / W1_SCALE)
                        ths.append((q0, ql, th))

                    if pend_mm2 is not None:
                        emit_mm2_and_E(pend_mm2)
                    pend_mm2 = (g, s, l, ths)

                    if g == ngroups // 2 and s == 0:
                        # first half of the e dump (tiles of groups
                        # 0..mid-1 are final), overlapped with compute.
                        # Issued from the ACT sequencer: on the Sync/DGE
                        # queue its wait-for-exp would head-of-line block
                        # the input DMA issues behind it.
                        t = (ngroups // 2) * GT
                        nc.scalar.dma_start(e_out_d[:, :t], e_buf[:, :t])
                    if s == 16 and ready_pool:
                        emit_group_pool(ready_pool.pop(0))

            emit_mm2_and_E(pend_mm2)
            while ready_pool:
                emit_group_pool(ready_pool.pop(0))
            emit_scat(flush=True)

            pool_sb = outp.tile([SEGS_PER_CORE, D], F32)
            nc.scalar.activation(pool_sb[:], pool_ps[:], AF.Copy)
            nc.default_dma_engine.dma_start(out_d[:], pool_sb[:])
            t = (ngroups // 2) * GT
            nc.default_dma_engine.dma_start(e_out_d[:, t:], e_buf[:, t:])

    nc.compile()
    return nc


def _prep_core(x, batch, r0, r1, seg0, nt):
    """Host-side shard prep for one core: rows [r0, r1) own segments
    [seg0, seg0+128). Returns the per-core input map."""
    rows = r1 - r0
    pad_rows = nt * 128

    xb = np.zeros((pad_rows, D), dtype=f16)
    xb[:rows] = x[r0:r1].astype(f16)
    # (128, nt, D): partition p holds row t*128 + p
    x_nat = np.ascontiguousarray(xb.reshape(nt, 128, D).transpose(1, 0, 2))

    x8 = np.zeros((pad_rows, D), dtype=f8)
    x8[:rows] = x[r0:r1].astype(f8)
    # (128, ngroups, 2, GT*128): partition d' holds feature c*128 + d',
    # packed so each group's slice is one contiguous line per partition
    xT = np.ascontiguousarray(
        x8.T.reshape(2, 128, nt // GT, GT * 128).transpose(1, 2, 0, 3))

    seg_local = np.full(pad_rows, -1, dtype=np.int64)
    seg_local[:rows] = batch[r0:r1] - seg0
    ci = np.where(seg_local < 0, -1.0, seg_local % EW).astype(np.float32)
    colidx = np.ascontiguousarray(ci.reshape(nt, 128).T).astype(f16)  # (128, nt)

    # scatter matrices: scat[k, g, s] = 1 iff group g's pool row k holds
    # local segment s (k = s mod EW). A 31-tile group spans <= ~10
    # consecutive segments, so within a group mod-EW is collision free.
    ngroups = nt // GT
    scat = np.zeros((EW, ngroups, SEGS_PER_CORE), dtype=f16)
    for g in range(ngroups):
        segs = np.unique(seg_local[g * GT * 128:(g + 1) * GT * 128])
        segs = segs[segs >= 0]
        assert segs.size <= EW, f"group {g} spans {segs.size} segments > EW"
        scat[segs % EW, g, segs] = 1.0

    return {"x_nat": x_nat, "xT": xT, "colidx": colidx, "scat": scat}


def _shared_inputs(W1, W2):
    W1s = (W1_SCALE * W1).astype(np.float32)
    A = W1s.astype(f8)
    Bm = (W1s - A.astype(np.float32)).astype(f8)

    if SWI:
        def pack(w):
            # (H, D) -> (128, 2H) flat: flat[p, 2*(127-h)+c] = w[h, c*128+p]
            Wc = np.asarray(w).T.reshape(2, 128, H)  # [c, p, h]
            flat = np.zeros((128, 2 * H), dtype=w.dtype)
            h = np.arange(H)
            for c in range(2):
                flat[:, 2 * (H - 1 - h) + c] = Wc[c]
            return np.ascontiguousarray(flat)
    else:
        def pack(w):  # (H, D) -> (128, 2, H) with [d', c, h] = w[h, c*128+d']
            return np.ascontiguousarray(
                np.asarray(w).T.reshape(2, 128, H).transpose(1, 0, 2))

    w2c = np.ascontiguousarray(W2.reshape(H, 1).astype(f16))
    iota = np.broadcast_to(
        np.arange(EW, dtype=np.float32), (128, EW)).astype(f16)
    return {"W1A": pack(A), "W1B": pack(Bm), "W2c": w2c, "iota": iota}


def _seg_starts(x, batch):
    s = np.searchsorted(batch, np.arange(0, NCORES * SEGS_PER_CORE + 1, SEGS_PER_CORE))
    s[0], s[-1] = 0, x.shape[0]
    return s


def build_in_maps(x, batch, nt):
    s = _seg_starts(x, batch)
    return [_prep_core(x, batch, int(s[c]), int(s[c + 1]), c * SEGS_PER_CORE, nt)
            for c in range(NCORES)]


def pick_nt(x, batch):
    s = _seg_starts(x, batch)
    nt = int(max(-(-(int(s[c + 1] - s[c])) // 128) for c in range(NCORES)))
    return -(-nt // GT) * GT


def kernel(x, batch, W1, W2, B):
    x = np.asarray(x)
    batch = np.asarray(batch)
    W1 = np.asarray(W1)
    W2 = np.asarray(W2)
    B = int(B)
    assert B == NCORES * SEGS_PER_CORE

    nt = pick_nt(x, batch)
    if nt not in _kernel_cache:
        _kernel_cache[nt] = _build_kernel(nt)
    nc = _kernel_cache[nt]

    shared = _shared_inputs(W1, W2)
    in_maps = build_in_maps(x, batch, nt)
    for m in in_maps:
        m.update(shared)

    res = run_bass_kernel_spmd(nc, in_maps, core_ids=list(range(NCORES)))

    seg_starts = _seg_starts(x, batch)
    z = np.empty((B, D), dtype=np.float32)
    for c in range(NCORES):
        num = res.results[c]["out"]  # (128, D)
        # denominator from the e dump, rounded exactly like the E matrix
        e = res.results[c]["e_out"].T.reshape(-1)  # row t*128+p -> e
        r0, r1 = int(seg_starts[c]), int(seg_starts[c + 1])
        seg_local = (batch[r0:r1] - c * SEGS_PER_CORE).astype(np.int64)
        e_rows = e[:r1 - r0].astype(np.float64)
        den = np.bincount(seg_local, weights=e_rows, minlength=SEGS_PER_CORE)
        den = np.where(den == 0.0, 1.0, den).astype(np.float32)
        z[c * SEGS_PER_CORE:(c + 1) * SEGS_PER_CORE] = num / den[:, None]
    return z
